# revision 21
# baseline (speedup 1.0000x reference)
"""Trainium2 Bass kernel for nn_DisentangledSelfAttentionWeighted.

Reference math (per sample, L=128, E=A=256, H=4, D=64):
    q = xq@Wq+bq, k = xk@Wk+bk, v = xv@Wv+bv, qp = xq@Ww+bw   (H heads of D)
    pair  = softmax_m( (q-mean_l q) . (k-mean_l k) )          [H,L,L]
    un    = softmax_m( k . mean_l qp )                        [H,L]
    out   = (pair + un) @ v + xq@Wr + br

Algebraic restructuring (exact up to fp rounding):
  * softmax over m drops anything constant over m:
      pair logits == (q0_l - mu_q) . k0_m   (all biases drop; only Q needs
      centering, done IN PSUM via per-sample rank-1 matmul updates
      q_ps -= mu_q ⊗ 1 so the exp needs no bias and batches freely)
      un logits   == k0_m . mu_qp,  mu_qp = Ww^T mean_l(xq) + bw
  * sum_m softmax == 1 twice, so bv enters twice: out += br + 2*bv via a
    rank-1 ones4 broadcast matmul folded into the residual accumulation.
  * softmax normalization deferred: unnormalized exp goes through the
    attention@V matmul; a ones column in V yields the row denominators,
    applied per (l, head) during the output combine.

Per-group (g samples) batching:
  * inputs DMA'd [L, g, E] f32, cast to bf16 on the (otherwise idle)
    gpsimd engine, transposed with ONE batched DMA-xbar transpose per
    input ([l,(s,c,e)] -> [e,(s,c),l]) -- no PSUM, no PE time  (mode
    "dmax"); or fp32 PE transpose-mode straight from the f32 input
    (mode "pe32").
  * q/k projections W-stationary with N=g*L=512 moving columns; v/r
    projections x-stationary N=256.
  * all PSUM evacuations batched to [128, 512]-size ops, split between
    the Activation and DVE engines; gpsimd handles SBUF-only work.
"""

import os
import sys
from contextlib import ExitStack

import numpy as np

sys.path.insert(0, "/opt/trn_rl_repo")

import ml_dtypes  # noqa: E402
import concourse.bass as bass  # noqa: E402
import concourse.tile as tile  # noqa: E402
from concourse import mybir  # noqa: E402

BF16 = mybir.dt.bfloat16
F32 = mybir.dt.float32
AF = mybir.ActivationFunctionType
ALU = mybir.AluOpType

B, L, E, A, H = 1024, 128, 256, 256, 4
D = A // H
NCORES = 8
NB = B // NCORES  # samples per core
G = 4  # samples per group

# "dmax": DMA-xbar transpose of bf16 inputs (cast on gpsimd)
# "pe32": fp32 transpose-mode on PE straight from the f32 input
MODE = os.environ.get("KERNEL_MODE", "pe32")


def _prep_w(w):
    # [E, A] f32 -> [128, 2, A] bf16  (partition = e_lo within chunk)
    return np.ascontiguousarray(
        w.reshape(2, 128, A).transpose(1, 0, 2).astype(ml_dtypes.bfloat16)
    )


def prep_consts(Wq, bq, Wk, bk, Wv, bv, Ww, bw, Wr, br):
    """Host-side constant preparation (shared across cores)."""
    consts = {
        "WqB": _prep_w(Wq),
        "WkB": _prep_w(Wk),
        "WvB": _prep_w(Wv),
        "WwB": _prep_w(Ww),
        "WrB": _prep_w(Wr),
        # bw as per-partition columns for the two A-chunks
        "bw2": np.ascontiguousarray(bw.reshape(2, 128).T.astype(np.float32)),
        # (br + 2*bv)/4 replicated over 4 partitions (summed by ones4 matmul)
        "brbv4": np.ascontiguousarray(
            np.tile((br + 2.0 * bv)[None, :] / 4.0, (4, 1)).astype(ml_dtypes.bfloat16)
        ),
        "ones4": np.ones((4, L), dtype=ml_dtypes.bfloat16),
        # mask4[h', 64h+j] = (h'==h): picks diagonal blocks of the uv product
        "mask4": np.kron(np.eye(4), np.ones((1, D))).astype(ml_dtypes.bfloat16),
        # selL[s', 128s+l] = (s'==s): block-selector for the rank-1 centering
        "selL": np.kron(np.eye(4), np.ones((1, L))).astype(ml_dtypes.bfloat16),
    }
    return consts


def split_excess_waits(nc):
    """Walrus/ISA allows ONE sync wait per engine instruction; Tile sometimes
    emits more.  Move excess waits onto same-engine NOPs inserted just before
    the offending instruction (engine queues execute in program order)."""
    import bass_rust

    engmap = {e.engine: e for e in nc.engines.values()}
    for f in nc.m.functions:
        for b in f.blocks:
            il = b.instructions
            idx = 0
            while idx < len(il):
                inst = il[idx]
                si = inst.sync_info
                if (
                    si is not None
                    and len(si.on_wait) > 1
                    and type(inst).__name__ not in ("InstEventSemaphore",)
                ):
                    waits = list(si.on_wait)
                    keep = waits[-1]
                    inst.sync_info = bass_rust.SyncInfo(
                        on_wait=[keep], on_update=list(si.on_update)
                    )
                    for w in waits[:-1]:
                        nop_bi = engmap[inst.engine].nop()
                        nop = nop_bi.ins
                        for bb in nc.m.functions[0].blocks:
                            lst = bb.instructions
                            if lst and lst[-1] is nop:
                                lst.pop()
                                break
                        nop.sync_info = bass_rust.SyncInfo(
                            on_wait=[w], on_update=[]
                        )
                        il.insert(idx, nop)
                        idx += 1
                idx += 1


def build_nc(nb=NB, g=G, mode=MODE, repeats=1, upto="full"):
    """Build the per-core Bass program for nb samples.

    repeats>1 re-runs the whole computation (same I/O) for timing: the
    difference between repeats=2 and repeats=1 wall-clock is one pass of
    pure device execution with transfer/dispatch overheads cancelled.
    """
    assert nb % g == 0
    assert g == 4
    ngrp = nb // g
    nc = bass.Bass("TRN2", debug=False)

    # ---- DRAM I/O ----
    dq = nc.dram_tensor("query", [nb, L, E], F32, kind="ExternalInput").ap()
    dk = nc.dram_tensor("key", [nb, L, E], F32, kind="ExternalInput").ap()
    dv = nc.dram_tensor("value", [nb, L, E], F32, kind="ExternalInput").ap()
    dW = {
        n: nc.dram_tensor(n, [128, 2, A], BF16, kind="ExternalInput").ap()
        for n in ("WqB", "WkB", "WvB", "WwB", "WrB")
    }
    dbw2 = nc.dram_tensor("bw2", [128, 2], F32, kind="ExternalInput").ap()
    dbrbv4 = nc.dram_tensor("brbv4", [4, A], BF16, kind="ExternalInput").ap()
    dones4 = nc.dram_tensor("ones4", [4, L], BF16, kind="ExternalInput").ap()
    dmask4 = nc.dram_tensor("mask4", [4, A], BF16, kind="ExternalInput").ap()
    dselL = nc.dram_tensor("selL", [4, 4 * L], BF16, kind="ExternalInput").ap()
    dout = nc.dram_tensor("out", [nb, L, A], F32, kind="ExternalOutput").ap()

    with tile.TileContext(nc) as tc, ExitStack() as ctx:
        const = ctx.enter_context(tc.tile_pool(name="const", bufs=1))
        xin = ctx.enter_context(tc.tile_pool(name="xin", bufs=3))
        xbfp = ctx.enter_context(tc.tile_pool(name="xbf", bufs=2))
        xtp = ctx.enter_context(tc.tile_pool(name="xt", bufs=2))
        projp = ctx.enter_context(tc.tile_pool(name="proj", bufs=2))
        smallp = ctx.enter_context(tc.tile_pool(name="small", bufs=2))
        expp = ctx.enter_context(tc.tile_pool(name="expp", bufs=3))
        outp = ctx.enter_context(tc.tile_pool(name="outs", bufs=2))
        # PSUM: 8 banks total.
        ps_qkv = ctx.enter_context(tc.tile_pool(name="ps_qkv", bufs=2, space="PSUM"))
        ps_pair = ctx.enter_context(tc.tile_pool(name="ps_pair", bufs=2, space="PSUM"))
        if mode == "pe32":
            ps_tp = ctx.enter_context(tc.tile_pool(name="ps_tp", bufs=2, space="PSUM"))
            ps_avz = ctx.enter_context(
                tc.tile_pool(name="ps_avz", bufs=2, space="PSUM")
            )
            ps_r = ps_avz
        else:
            ps_avz = ctx.enter_context(
                tc.tile_pool(name="ps_avz", bufs=2, space="PSUM")
            )
            ps_r = ctx.enter_context(tc.tile_pool(name="ps_r", bufs=2, space="PSUM"))

        # ---- constants to SBUF ----
        W = {}
        for n in ("WqB", "WkB", "WvB", "WwB", "WrB"):
            W[n] = const.tile([128, 2, A], BF16, name=n + "_sb")
            nc.sync.dma_start(out=W[n], in_=dW[n])
        bw_sb = const.tile([128, 2], F32, name="bw_sb")
        nc.sync.dma_start(out=bw_sb, in_=dbw2)
        brbv_sb = const.tile([4, A], BF16, name="brbv_sb")
        nc.sync.dma_start(out=brbv_sb, in_=dbrbv4)
        ones4_sb = const.tile([4, L], BF16, name="ones4_sb")
        nc.sync.dma_start(out=ones4_sb, in_=dones4)
        mask4_sb = const.tile([4, A], BF16, name="mask4_sb")
        nc.sync.dma_start(out=mask4_sb, in_=dmask4)
        selL_sb = const.tile([4, 4 * L], BF16, name="selL_sb")
        nc.sync.dma_start(out=selL_sb, in_=dselL)
        if mode == "pe32":
            from concourse.masks import make_identity

            ident = const.tile([128, 128], F32, name="ident")
            make_identity(nc, ident)

        for grp in range(ngrp * repeats):
            grp = grp % ngrp
            s0 = grp * g
            # ---- load inputs (DRAM [g, L, E] -> SBUF [L, g, E]) ----
            q_nat = xin.tile([L, g, E], F32, tag="qnat", name="q_nat")
            k_nat = xin.tile([L, g, E], F32, tag="knat", name="k_nat")
            v_nat = xin.tile([L, g, E], F32, tag="vnat", name="v_nat")
            nc.sync.dma_start(out=q_nat, in_=dq[s0 : s0 + g].rearrange("g l e -> l g e"))
            nc.sync.dma_start(out=k_nat, in_=dk[s0 : s0 + g].rearrange("g l e -> l g e"))
            nc.sync.dma_start(out=v_nat, in_=dv[s0 : s0 + g].rearrange("g l e -> l g e"))

            # transposed bf16 inputs: xT[n] = [e_lo(128), s, c, l]
            xT = {
                n: xtp.tile([128, g, 2, L], BF16, tag=f"{n}T", name=f"{n}T")
                for n in ("q", "k", "v")
            }
            if mode == "dmax":
                # cast on gpsimd, then ONE batched xbar transpose per input:
                # in [l, (s,c,e_lo)] -> out [e_lo, (s,c), l]
                for n, nat in (("q", q_nat), ("k", k_nat), ("v", v_nat)):
                    xbf = xbfp.tile([L, g, E], BF16, tag=f"{n}bf", name=f"{n}bf")
                    nc.gpsimd.tensor_copy(xbf, nat)
                    nc.scalar.dma_start_transpose(out=xT[n], in_=xbf)
            else:
                # fp32 PE transpose-mode; evac (with bf16 cast) split Act/DVE
                for n, nat in (("q", q_nat), ("k", k_nat), ("v", v_nat)):
                    for c in range(2):
                        tp_ps = ps_tp.tile([128, g, 128], F32, tag="tp", name="tp_ps")
                        for s in range(g):
                            nc.tensor.transpose(
                                tp_ps[:, s, :],
                                nat[:, s, c * 128 : (c + 1) * 128],
                                ident,
                            )
                        eng = nc.scalar if (c == 0) else nc.vector
                        if eng is nc.scalar:
                            nc.scalar.copy(xT[n][:, :, c, :], tp_ps)
                        else:
                            nc.vector.tensor_copy(xT[n][:, :, c, :], tp_ps)

            # ---- means of xq over l (gpsimd; SBUF-only) ----
            mq_f = smallp.tile([128, g, 2], F32, tag="mq_f", name="mq_f")
            nc.vector.tensor_reduce(
                mq_f.rearrange("p s c -> p (s c)"),
                xT["q"].rearrange("p s c l -> p (s c) l"),
                axis=mybir.AxisListType.X,
                op=ALU.add,
            )
            mq_neg = smallp.tile([128, g, 2], BF16, tag="mq_neg", name="mq_neg")
            mq_pos = smallp.tile([128, g, 2], BF16, tag="mq_pos", name="mq_pos")
            nc.gpsimd.tensor_scalar(mq_neg, mq_f, -1.0 / L, None, op0=ALU.mult)
            nc.gpsimd.tensor_scalar(mq_pos, mq_f, 1.0 / L, None, op0=ALU.mult)

            # ---- mu matmuls (allocate mu bank FIRST: see pool-cycle note) ----
            # mu_t regions: [:, 0, 0:8] = mu_qp [a_lo, (ac, s)] (f32)
            #               [0:4, 2:4, :] = -mu_q^T [s, a]      (f32)
            mu_t = ps_qkv.tile([128, 4, 128], F32, tag="qkv", name="mu_t")
            muqp_ps = mu_t[:, 0, 0:8].rearrange("p (ac s) -> p ac s", ac=2)
            muT_ps = mu_t[0:4, 2:4, :]
            for c in range(2):
                nc.tensor.matmul(
                    muT_ps,
                    lhsT=mq_neg[:, :, c],
                    rhs=W["WqB"][:, c, :],
                    start=(c == 0),
                    stop=(c == 1),
                )
            for ac in range(2):
                for c in range(2):
                    nc.tensor.matmul(
                        muqp_ps[:, ac, :],
                        lhsT=W["WwB"][:, c, ac * 128 : (ac + 1) * 128],
                        rhs=mq_pos[:, :, c],
                        start=(c == 0),
                        stop=(c == 1),
                    )
            muT_sb = smallp.tile([4, 2, 128], BF16, tag="muT", name="muT_sb")
            nc.vector.tensor_copy(muT_sb, muT_ps)

            # ---- q/k projections (W stationary, N=512) + rank-1 centering ----
            # QP[ac]: [a_lo, s, 130]: cols 0:128 = centered qT, col 128 = mu_qp
            QP = [projp.tile([128, g, 130], BF16, tag=f"QP{ac}", name=f"QP{ac}")
                  for ac in range(2)]
            KP = [projp.tile([128, g, L], BF16, tag=f"KP{ac}", name=f"KP{ac}")
                  for ac in range(2)]
            V = projp.tile([L, g, H, D + 1], BF16, tag="V", name="V")

            q_ps = []
            for ac in range(2):
                ps = ps_qkv.tile([128, 512], F32, tag="qkv", name=f"q_ps{ac}")
                q_ps.append(ps)
                for c in range(2):
                    nc.tensor.matmul(
                        ps,
                        lhsT=W["WqB"][:, c, ac * 128 : (ac + 1) * 128],
                        rhs=xT["q"][:, :, c, :],
                        start=(c == 0),
                        stop=False,
                    )
            k_ps = []
            for ac in range(2):
                ps = ps_qkv.tile([128, 512], F32, tag="qkv", name=f"k_ps{ac}")
                k_ps.append(ps)
                for c in range(2):
                    nc.tensor.matmul(
                        ps,
                        lhsT=W["WkB"][:, c, ac * 128 : (ac + 1) * 128],
                        rhs=xT["k"][:, :, c, :],
                        start=(c == 0),
                        stop=(c == 1),
                    )
            v_ps = []
            for half in range(2):
                ps = ps_qkv.tile([L, 2, 256], F32, tag="qkv", name=f"v_ps{half}")
                v_ps.append(ps)
                for s2 in range(2):
                    s = half * 2 + s2
                    for c in range(2):
                        nc.tensor.matmul(
                            ps[:, s2, :],
                            lhsT=xT["v"][:, s, c, :],
                            rhs=W["WvB"][:, c, :],
                            start=(c == 0),
                            stop=(c == 1),
                        )
            # rank-1 centering for all samples at once:
            # q_ps[:, 128s+l] += sum_{s'} (-mu_q[s',a]) * selL[s', 128s+l]
            for ac in range(2):
                nc.tensor.matmul(
                    q_ps[ac],
                    lhsT=muT_sb[:, ac, :],
                    rhs=selL_sb,
                    start=False,
                    stop=True,
                )

            # ---- evacuations ----
            # QP col 128 = mu_qp + bw  (DVE, reads mu PSUM)
            for ac in range(2):
                nc.vector.tensor_scalar(
                    QP[ac][:, :, 128:129],
                    muqp_ps[:, ac, :].unsqueeze(2),
                    bw_sb[:, ac : ac + 1],
                    None,
                    op0=ALU.add,
                )
            for ac in range(2):
                nc.scalar.copy(
                    QP[ac][:, :, 0:128],
                    q_ps[ac].rearrange("p (s l) -> p s l", s=g),
                )
            for ac in range(2):
                nc.scalar.copy(
                    KP[ac],
                    k_ps[ac].rearrange("p (s l) -> p s l", s=g),
                )
            nc.gpsimd.memset(V[:, :, :, D : D + 1], 1.0)
            for half in range(2):
                nc.scalar.copy(
                    V[:, 2 * half : 2 * half + 2, :, 0:D],
                    v_ps[half].rearrange("p s (h d) -> p s h d", h=H),
                )

            out_sb = outp.tile([L, g, A], F32, tag="out_sb", name="out_sb")

            if upto == "proj":
                nc.vector.tensor_copy(out_sb[:, :, 0:L], KP[0])
                nc.vector.tensor_copy(out_sb[:, :, L:2*L], QP[1][:, :, 0:128])
                nc.sync.dma_start(
                    out=dout[s0 : s0 + g].rearrange("g l a -> l g a"), in_=out_sb
                )
                continue

            # ---- per-sample attention, software-pipelined by one sample ----
            pair_tiles = [None] * g
            expT_tiles = [None] * g

            def emit_pair(s):
                # pair logits (bias-free): pairT[m, l] = k0T_h . qcT_h
                # Two heads per bank, grouped by SAME partition base (hh):
                # bank hh holds heads {hh, hh+2} (slots ac=0,1).  Mixing
                # partition bases within one PSUM bank wedges the PE.
                tiles = []
                for hh in range(2):
                    off = hh * 64
                    pp = ps_pair.tile([128, 2, 130], F32, tag="pair",
                                      name=f"pair{hh}")
                    tiles.append(pp)
                    for ac in range(2):
                        nc.tensor.matmul(
                            pp[:, ac, 0:129],
                            lhsT=KP[ac][off : off + 64, s, :],
                            rhs=QP[ac][off : off + 64, s, 0:129],
                            start=True,
                            stop=True,
                        )
                pair_tiles[s] = tiles

            def emit_exp(s):
                # head h = 2*ac + hh lives in bank hh, slot ac ->
                # expT[:, h] with h = 2*ac + hh: bank hh covers heads hh, hh+2
                expT = expp.tile([128, H, 130], BF16, tag="expT", name="expT")
                expT_tiles[s] = expT
                eview = expT.rearrange("p (ac hh) c -> p hh ac c", ac=2)
                for hh in range(2):
                    nc.scalar.activation(
                        eview[:, hh, :, 0:129],
                        pair_tiles[s][hh][:, :, 0:129],
                        AF.Exp,
                    )

            def emit_tail(s):
                expT = expT_tiles[s]
                # uv cross products + Zu (from V's ones cols), all 4 heads
                uvz = ps_avz.tile([4, H, D + 1], F32, tag="avz", name="uvz")
                nc.tensor.matmul(
                    uvz,
                    lhsT=expT[:, :, 128],
                    rhs=V[:, s, :, :].rearrange("p h d -> p (h d)"),
                    start=True,
                    stop=True,
                )
                # attention @ V (ones col -> Zp per row)
                av = ps_avz.tile([L, H, D + 1], F32, tag="avz", name="av")
                for h in range(H):
                    nc.tensor.matmul(
                        av[:, h, :],
                        lhsT=expT[:, h, 0:128],
                        rhs=V[:, s, h, :],
                        start=True,
                        stop=True,
                    )
                ruz = smallp.tile([4, 1], F32, tag="ruz", name="ruz")
                nc.vector.reciprocal(ruz, uvz[:, 0, D : D + 1])
                uvb = smallp.tile([4, H, D], BF16, tag="uvb", name="uvb")
                nc.vector.scalar_tensor_tensor(
                    uvb,
                    uvz[:, :, 0:D],
                    ruz,
                    mask4_sb.rearrange("p (h d) -> p h d", h=H),
                    op0=ALU.mult,
                    op1=ALU.mult,
                )
                # residual r = xq@Wr + ones4 ⊗ (uv + brbv)
                r_ps = ps_r.tile([L, A], F32, tag=("avz" if mode == "pe32" else "r"),
                                 name="r_ps")
                for c in range(2):
                    nc.tensor.matmul(
                        r_ps,
                        lhsT=xT["q"][:, s, c, :],
                        rhs=W["WrB"][:, c, :],
                        start=(c == 0),
                        stop=False,
                    )
                nc.tensor.matmul(
                    r_ps, lhsT=ones4_sb, rhs=uvb.rearrange("p h d -> p (h d)"),
                    start=False, stop=False,
                )
                nc.tensor.matmul(
                    r_ps, lhsT=ones4_sb, rhs=brbv_sb, start=False, stop=True
                )
                # normalize + combine
                rzp = smallp.tile([128, H], F32, tag="rzp", name="rzp")
                nc.vector.reciprocal(rzp, av[:, :, D])
                nc.vector.tensor_tensor(
                    out_sb[:, s, :].rearrange("p (h d) -> p h d", h=H),
                    av[:, :, 0:D],
                    rzp.unsqueeze(2).broadcast_to((128, H, D)),
                    op=ALU.mult,
                )
                nc.vector.tensor_tensor(
                    out_sb[:, s, :], out_sb[:, s, :], r_ps, op=ALU.add
                )

            def emit_exp_only_out(s):
                nc.vector.tensor_copy(
                    out_sb[:, s, :].rearrange("p (h d) -> p h d", h=H),
                    expT_tiles[s][:, :, 0:64],
                )

            def emit_av_only(s):
                expT = expT_tiles[s]
                av = ps_avz.tile([L, H, D + 1], F32, tag="avz", name="av")
                for h in range(H):
                    nc.tensor.matmul(
                        av[:, h, :],
                        lhsT=expT[:, h, 0:128],
                        rhs=V[:, s, h, :],
                        start=True,
                        stop=True,
                    )
                rzp = smallp.tile([128, H], F32, tag="rzp", name="rzp")
                nc.vector.reciprocal(rzp, av[:, :, D])
                for h in range(H):
                    nc.vector.scalar_tensor_tensor(
                        out_sb[:, s, h * D : (h + 1) * D],
                        av[:, h, 0:D],
                        rzp[:, h : h + 1],
                        out_sb[:, s, h * D : (h + 1) * D],
                        op0=ALU.mult,
                        op1=ALU.mult,
                    )

            emit_pair(0)
            for s in range(g):
                if s + 1 < g:
                    emit_pair(s + 1)
                emit_exp(s)
                if upto == "exp":
                    emit_exp_only_out(s)
                elif upto == "av":
                    emit_av_only(s)
                else:
                    emit_tail(s)

            nc.sync.dma_start(
                out=dout[s0 : s0 + g].rearrange("g l a -> l g a"), in_=out_sb
            )
    split_excess_waits(nc)
    return nc


_NC_CACHE = {}


def _get_nc(nb=NB):
    if nb not in _NC_CACHE:
        _NC_CACHE[nb] = build_nc(nb)
    return _NC_CACHE[nb]


def kernel(query, key, value, Wq, bq, Wk, bk, Wv, bv, Ww, bw, Wr, br):
    query = np.asarray(query, dtype=np.float32)
    key = np.asarray(key, dtype=np.float32)
    value = np.asarray(value, dtype=np.float32)
    consts = prep_consts(
        np.asarray(Wq, np.float32), np.asarray(bq, np.float32),
        np.asarray(Wk, np.float32), np.asarray(bk, np.float32),
        np.asarray(Wv, np.float32), np.asarray(bv, np.float32),
        np.asarray(Ww, np.float32), np.asarray(bw, np.float32),
        np.asarray(Wr, np.float32), np.asarray(br, np.float32),
    )
    nc = _get_nc(NB)
    from concourse.bass_utils import run_bass_kernel_spmd

    in_maps = []
    for core in range(NCORES):
        sl = slice(core * NB, (core + 1) * NB)
        m = {"query": query[sl], "key": key[sl], "value": value[sl]}
        m.update(consts)
        in_maps.append(m)
    res = run_bass_kernel_spmd(nc, in_maps, core_ids=list(range(NCORES)))
    out = np.concatenate([r["out"] for r in res.results], axis=0)
    return out.astype(np.float32)


if __name__ == "__main__":
    rng = np.random.default_rng(0)
    s = 0.02
    inputs = {
        "query": rng.standard_normal((B, L, E), dtype=np.float32),
        "key": rng.standard_normal((B, L, E), dtype=np.float32),
        "value": rng.standard_normal((B, L, E), dtype=np.float32),
    }
    for n in ("q", "k", "v", "w", "r"):
        inputs["W" + n] = rng.standard_normal((E, A), dtype=np.float32) * s
        inputs["b" + n] = rng.standard_normal((A,), dtype=np.float32) * s
    out = kernel(**inputs)
    print("out", out.shape, out.dtype, float(np.abs(out).max()))
